# revision 41
# baseline (speedup 1.0000x reference)
"""MHA kernel for Trainium2, 8 NeuronCores — v2.

Sharding: core c -> batch b = c//2, head-block hb = c%2 (8 of 16 heads).
Tensor-parallel within a batch: Wq/Wk/Wv column-sliced, Wo row-sliced;
each core emits a partial output [2048, 1024]; host sums the two partials
per batch and adds the bias.

v2 changes vs v1 (508.9us):
  - host pre-transposes x (xT supplied directly) and pre-casts all
    operands to bf16: kills the on-device transpose phase and halves DMA.
  - PV computed row-major: O[q,d] via lhsT=P^T[kv,q-block] (K=kv=128 full
    contraction) instead of v1's OT[d,q] (K=65): halves PV PE time. The
    denominator rides along as a ones-column appended per-head to V
    (65th output col). O is then normalized per-partition (q) with a DVE
    tensor_scalar using the broadcast reciprocal of col 64 — no bc-matmul.
  - O transposed on PE afterwards (cheap) for the Wo projection.
  - all non-attention PE work (KT/QT/V projections, O transposes, out
    projection) runs as a task queue drained in the S->exp pipeline's PE
    slack so the Activation engine (softmax exp, ~266us) never starves.

Engine budgets (TimelineSim cost model): PE ~277us busy, ACT ~266us.
"""
from collections import deque

import numpy as np
import ml_dtypes

import concourse.bacc as bacc
import concourse.mybir as mybir
from concourse.tile import TileContext
from concourse.bass_utils import run_bass_kernel_spmd

F32 = mybir.dt.float32
BF16 = mybir.dt.bfloat16
AF = mybir.ActivationFunctionType
MUL = mybir.AluOpType.mult

N = 2048       # tokens per batch
DIM = 1024     # model dim
HL = 512       # local inner (8 heads x 64)
NK = 8         # dim contraction 128-tiles
NJ = 16        # kv 128-tiles
NQT = 4        # q 512-tiles
NHP = 4        # local head pairs

PT_BUFS = 26   # exp->PV elasticity (tiles of [128,1024] bf16)
BUDGET = 620   # ns of filler PE work drained per attention iter
BUDGET0 = 800  # drain budget during qt0 (projection backlog phase)

_CACHE = {}


def build():
    nc = bacc.Bacc(None, target_bir_lowering=False)
    xT_d = nc.declare_dram_parameter("xT", [DIM, N], BF16, isOutput=False)
    # weights pre-concatenated on host into one 128-partition row so each
    # loads with a single DMA (the shared HWDGE charges ~625ns per DMA)
    wq_d = nc.declare_dram_parameter("wq", [128, NK * HL], BF16, isOutput=False)
    wk_d = nc.declare_dram_parameter("wk", [128, NK * HL], BF16, isOutput=False)
    wv_d = nc.declare_dram_parameter("wv", [128, NK * HL], BF16, isOutput=False)
    wo_d = nc.declare_dram_parameter("wo", [128, 4 * DIM], BF16, isOutput=False)
    ones_d = nc.declare_dram_parameter("ones", [128, 8], BF16, isOutput=False)
    ident_d = nc.declare_dram_parameter("ident", [128, 128], BF16,
                                        isOutput=False)
    out_d = nc.declare_dram_parameter("out", [N, DIM], F32, isOutput=True)

    with TileContext(nc) as tc:
        with (
            tc.tile_pool(name="sb", bufs=1) as sb,
            tc.tile_pool(name="ps", bufs=2, space="PSUM") as psp,
        ):
            ones_sb = sb.tile([128, 8], BF16, name="ones", tag="ones", bufs=1)
            ident = sb.tile([128, 128], BF16, name="ident", tag="id", bufs=1)

            wkt = sb.tile([128, NK * HL], BF16, name="wkt", tag="wkt", bufs=1)
            wqt = sb.tile([128, NK * HL], BF16, name="wqt", tag="wqt", bufs=1)
            wvt = sb.tile([128, NK * HL], BF16, name="wvt", tag="wvt", bufs=1)
            wot = sb.tile([128, 4 * DIM], BF16, name="wot", tag="wot", bufs=1)
            # x^T in 8x4 chunks [128, 512] so consumers only wait their chunk
            xt = [[sb.tile([128, 512], BF16, name=f"xt{k}_{c}", tag="xt",
                           bufs=32) for c in range(4)] for k in range(NK)]
            KT = [sb.tile([128, N], BF16, name=f"kt{m}", tag="kt", bufs=4)
                  for m in range(4)]
            QT = [sb.tile([128, N], BF16, name=f"qt{m}", tag="qt", bufs=4)
                  for m in range(4)]
            OT = [sb.tile([128, N], BF16, name=f"ot{m}", tag="ot", bufs=4)
                  for m in range(4)]
            V = [sb.tile([128, 520], BF16, name=f"v{t}", tag="v", bufs=NJ)
                 for t in range(NJ)]
            O = [sb.tile([128, HL], BF16, name=f"o{t}", tag="o", bufs=NJ)
                 for t in range(NJ)]

            # DMA priority order: what the prologue needs lands first.
            # Weights ride the gpsimd SWDGE path, parallel to SP's HWDGE.
            nc.sync.dma_start(out=ident[:], in_=ident_d[:])
            # halves so the first projection k-tiles start ~2.5us sooner
            nc.gpsimd.dma_start(out=wkt[:, 0:2048], in_=wk_d[:, 0:2048])
            nc.gpsimd.dma_start(out=wqt[:, 0:2048], in_=wq_d[:, 0:2048])
            nc.gpsimd.dma_start(out=wkt[:, 2048:4096], in_=wk_d[:, 2048:4096])
            nc.gpsimd.dma_start(out=wqt[:, 2048:4096], in_=wq_d[:, 2048:4096])
            for k in range(NK):
                nc.sync.dma_start(out=xt[k][0][:],
                                  in_=xT_d[k * 128:(k + 1) * 128, 0:512])
            nc.sync.dma_start(out=ones_sb[:], in_=ones_d[:])
            for k in range(NK):
                nc.sync.dma_start(out=xt[k][1][:],
                                  in_=xT_d[k * 128:(k + 1) * 128, 512:1024])
            # big weight transfers hog the shared DMA engines for ~3us each:
            # slot them behind the x chunks whose consumers run first
            nc.sync.dma_start(out=wvt[:], in_=wv_d[:])
            for c in range(2, 4):
                for k in range(NK):
                    nc.sync.dma_start(
                        out=xt[k][c][:],
                        in_=xT_d[k * 128:(k + 1) * 128, c * 512:(c + 1) * 512])
            nc.sync.dma_start(out=wot[:], in_=wo_d[:])

            # warm the PE p-state (0.65->2.4GHz ramp needs ~3us of
            # continuous work) with ident transposes while DMAs stream
            for _ in range(6):
                wtr = psp.tile([128, 1024], BF16, name="wtr", tag="pp",
                               bufs=2)
                for q in range(4):
                    nc.tensor.transpose(wtr[:, q * 128:(q + 1) * 128],
                                        ident[:], ident[:])

            pt_tiles = {}
            pv_read_cnt = [0]  # pt tiles fully consumed (8 PV mms emitted)

            # ---- filler task generators (yield = one PE op, value = ~ns)
            def g_kt(hp, nb):
                ps = psp.tile([128, 512], F32, name="pp", tag="pp", bufs=2)
                for k in range(NK):
                    nc.tensor.matmul(
                        ps[:],
                        wkt[:, k * HL + hp * 128:k * HL + (hp + 1) * 128],
                        xt[k][nb][:],
                        start=(k == 0), stop=(k == NK - 1))
                    yield 213
                nc.vector.tensor_copy(
                    out=KT[hp][:, nb * 512:(nb + 1) * 512], in_=ps[:])

            def g_qt(m, qt):
                ps = psp.tile([128, 512], F32, name="pp", tag="pp", bufs=2)
                for k in range(NK):
                    nc.tensor.matmul(
                        ps[:],
                        wqt[:, k * HL + m * 128:k * HL + (m + 1) * 128],
                        xt[k][qt][:],
                        start=(k == 0), stop=(k == NK - 1))
                    yield 213
                nc.vector.tensor_copy(
                    out=QT[m][:, qt * 512:(qt + 1) * 512], in_=ps[:])

            def g_v(t, hpg):
                # V projection for one head-pair (2 heads, 128 inner cols):
                # PV(qt,hp) only needs its own pair's V columns, so splitting
                # by pair takes most of V off qt0's critical path.
                ps = psp.tile([128, 512], F32, name="pp", tag="pp", bufs=2)
                for k in range(NK):
                    nc.tensor.matmul(
                        ps[:, 0:128],
                        xt[k][t // 4][:, (t % 4) * 128:(t % 4 + 1) * 128],
                        wvt[:, k * HL + hpg * 128:k * HL + (hpg + 1) * 128],
                        start=(k == 0), stop=(k == NK - 1))
                    yield 60
                base = hpg * 130
                nc.vector.tensor_copy(
                    out=V[t][:, base + 64:base + 130:65],
                    in_=ones_sb[:, 0:2])
                nc.vector.tensor_copy(
                    out=V[t][:, base:base + 130].rearrange(
                        "p (h c) -> p h c", c=65)[:, :, 0:64],
                    in_=ps[:, 0:128].rearrange("p (h c) -> p h c", c=64))

            def g_pv(qt, hp, shared=None):
                # PSUM zero regions are whole 2KB banks: interleaved
                # start/stop accumulation groups in one bank clobber each
                # other. Zero the banks with DVE instead and accumulate with
                # start=False (pure read-modify-write).
                oaE = psp.tile([128, 512], F32, name="oaE", tag="oaE", bufs=1)
                oaO = psp.tile([128, 512], F32, name="oaO", tag="oaO", bufs=1)
                nc.vector.memzero(oaE[:])
                nc.vector.memzero(oaO[:])
                for j in range(NJ):
                    if shared is not None:
                        shared["j"] = j
                        yield 0  # boundary: slices stop BEFORE j's reads
                    while (qt, hp, j) not in pt_tiles:
                        yield None  # exp for this j not emitted yet
                    pt = pt_tiles[(qt, hp, j)]
                    for h in (0, 1):
                        oacc = oaE if h == 0 else oaO
                        hl = hp * 2 + h
                        for qb in range(4):
                            nc.tensor.matmul(
                                oacc[:, qb * 65:(qb + 1) * 65],
                                pt[:, h * 512 + qb * 128:
                                   h * 512 + (qb + 1) * 128],
                                V[j][:, hl * 65:(hl + 1) * 65],
                                start=False, stop=False,
                                skip_group_check=True)
                            yield 30
                    pv_read_cnt[0] += 1
                recE = sb.tile([128, 4], F32, name="rcE", tag="rcE", bufs=2)
                recO = sb.tile([128, 4], F32, name="rcO", tag="rcO", bufs=2)
                nc.vector.reciprocal(recE[:], oaE[:, 64:260:65])
                nc.vector.reciprocal(recO[:], oaO[:, 64:260:65])
                for qb in range(4):
                    t = qt * 4 + qb
                    for h, (oacc, rec) in enumerate(((oaE, recE),
                                                     (oaO, recO))):
                        nc.vector.tensor_scalar(
                            O[t][:, hp * 128 + h * 64:hp * 128 + (h + 1) * 64],
                            oacc[:, qb * 65:qb * 65 + 64],
                            rec[:, qb:qb + 1], None, MUL)
                yield 0

            def g_tr(qt, m):
                tr = psp.tile([128, 1024], BF16, name="tr", tag="pp", bufs=2)
                for t4 in range(4):
                    nc.tensor.transpose(
                        tr[:, t4 * 128:(t4 + 1) * 128],
                        O[qt * 4 + t4][:, m * 128:(m + 1) * 128], ident[:])
                    yield 60
                nc.vector.tensor_copy(
                    out=OT[m][:, qt * 512:(qt + 1) * 512], in_=tr[:, 0:512])

            def g_out(qt, t4, dm):
                tt = qt * 4 + t4
                ps = psp.tile([128, 512], F32, name="pp", tag="pp", bufs=2)
                for kk in range(4):
                    nc.tensor.matmul(
                        ps[:], OT[kk][:, tt * 128:(tt + 1) * 128],
                        wot[:, kk * DIM + dm * 512:kk * DIM + (dm + 1) * 512],
                        start=(kk == 0), stop=(kk == 3))
                    yield 213
                st = sb.tile([128, 512], F32, name="st", tag="st", bufs=4)
                nc.vector.tensor_copy(out=st[:], in_=ps[:])
                nc.sync.dma_start(
                    out=out_d[tt * 128:(tt + 1) * 128,
                              dm * 512:(dm + 1) * 512], in_=st[:])

            # qt3 tail split: kk 0..2 accumulate early (needs only TR(3,0..2))
            # into a bf16 partial; only the kk=3 matmul + add + store remain
            # after the final TR.
            def g_out3a(t4, dm):
                tt = 12 + t4
                ps = psp.tile([128, 512], F32, name="pp", tag="pp", bufs=2)
                for kk in range(3):
                    nc.tensor.matmul(
                        ps[:], OT[kk][:, tt * 128:(tt + 1) * 128],
                        wot[:, kk * DIM + dm * 512:kk * DIM + (dm + 1) * 512],
                        start=(kk == 0), stop=(kk == 2))
                    yield 213
                pa = sb.tile([128, 512], BF16, name="pa", tag="pa", bufs=8)
                out3_part[(t4, dm)] = pa
                nc.vector.tensor_copy(out=pa[:], in_=ps[:])

            def g_out3b(t4, dm):
                tt = 12 + t4
                ps = psp.tile([128, 512], F32, name="pp", tag="pp", bufs=2)
                nc.tensor.matmul(
                    ps[:], OT[3][:, tt * 128:(tt + 1) * 128],
                    wot[:, 3 * DIM + dm * 512:3 * DIM + (dm + 1) * 512],
                    start=True, stop=True)
                yield 213
                st = sb.tile([128, 512], F32, name="st", tag="st", bufs=4)
                nc.vector.tensor_tensor(out=st[:], in0=ps[:],
                                        in1=out3_part[(t4, dm)][:],
                                        op=mybir.AluOpType.add)
                nc.sync.dma_start(
                    out=out_d[tt * 128:(tt + 1) * 128,
                              dm * 512:(dm + 1) * 512], in_=st[:])

            out3_part = {}

            tasks = deque()  # (key, generator)
            done = set()

            def drain(budget_ns):
                spent = 0
                while tasks and spent < budget_ns:
                    try:
                        cost = next(tasks[0][1])
                    except StopIteration:
                        done.add(tasks[0][0])
                        tasks.popleft()
                        continue
                    if cost is None:
                        break  # head task is waiting on future emissions
                    spent += cost

            def drain_until(key):
                # Emission-order barrier: everything a block reads must be
                # emitted before the block's reads, or Tile won't see the dep.
                while key not in done and tasks:
                    try:
                        cost = next(tasks[0][1])
                    except StopIteration:
                        done.add(tasks[0][0])
                        tasks.popleft()
                        continue
                    assert cost is not None, (
                        f"scheduling bug: waiting for {key} blocked behind "
                        f"stalled task {tasks[0][0]}")

            def run_now(g):
                for _ in g:
                    pass

            # ---- prologue: minimum PE work before the exp stream can start
            run_now(g_kt(0, 0))
            run_now(g_qt(0, 0))

            # ---- task queue in dependency order.
            # The pt ring (PT_BUFS) forces PV(0,0) to make progress early in
            # qt0, and PV(0,0,j) needs V[j]: interleave PV(0,0) j-slices
            # right behind each V(t) so exp never starves on pt slots longer
            # than the inherent projection backlog requires. KT/QT for the
            # next head-pairs are spread between so S never stalls either.
            def make_pv_sliced(qt, hp):
                shared = {"j": -1}
                shared["gen"] = g_pv(qt, hp, shared)

                def slice_(jmax):
                    while True:
                        if shared["j"] > jmax:
                            return
                        try:
                            cost = next(shared["gen"])
                        except StopIteration:
                            done.add(("pv", qt, hp))
                            return
                        yield cost
                return slice_

            g_pv00_slice = make_pv_sliced(0, 0)
            g_pv01_slice = make_pv_sliced(0, 1)

            for nb in range(1, 4):
                tasks.append((("kt", 0, nb), g_kt(0, nb)))
            tasks.append((("qt", 1, 0), g_qt(1, 0)))
            for nb in range(4):
                tasks.append((("kt", 1, nb), g_kt(1, nb)))
            for t in range(0, 4):
                tasks.append((("v", t, 0), g_v(t, 0)))
                tasks.append((("pvs", t), g_pv00_slice(t)))
            tasks.append((("qt", 2, 0), g_qt(2, 0)))
            for nb in range(4):
                tasks.append((("kt", 2, nb), g_kt(2, nb)))
            for t in range(4, NJ):
                tasks.append((("v", t, 0), g_v(t, 0)))
                tasks.append((("pvs", t), g_pv00_slice(t)))
            tasks.append((("pvs", 99), g_pv00_slice(NJ)))
            tasks.append((("tr", 0, 0), g_tr(0, 0)))
            # V pair-1 + first slices of PV(0,1) BEFORE KT(3): the pt-ring
            # guard needs PV(0,1) progress at exp idx 42, KT(3) only at 48.
            for t in range(NJ):
                tasks.append((("v", t, 1), g_v(t, 1)))
            tasks.append((("pvs1", 3), g_pv01_slice(3)))
            tasks.append((("qt", 3, 0), g_qt(3, 0)))
            for nb in range(4):
                tasks.append((("kt", 3, nb), g_kt(3, nb)))
            tasks.append((("pvs1", 99), g_pv01_slice(NJ)))
            tasks.append((("tr", 0, 1), g_tr(0, 1)))
            for hp in range(2, 4):
                for t in range(NJ):
                    tasks.append((("v", t, hp), g_v(t, hp)))
                tasks.append((("pv", 0, hp), g_pv(0, hp)))
                tasks.append((("tr", 0, hp), g_tr(0, hp)))
                if hp == 2:
                    for m in range(4):
                        tasks.append((("qt", m, 1), g_qt(m, 1)))
            for qt in range(1, NQT):
                for i, hp in enumerate(range(4)):
                    if qt + 1 < NQT and i == 1:
                        for m in range(4):
                            tasks.append((("qt", m, qt + 1),
                                          g_qt(m, qt + 1)))
                    if i == 2:
                        for t4 in range(4):
                            for dm in range(2):
                                tasks.append((("out", qt - 1, t4, dm),
                                              g_out(qt - 1, t4, dm)))
                    tasks.append((("pv", qt, hp), g_pv(qt, hp)))
                    tasks.append((("tr", qt, hp), g_tr(qt, hp)))
            for t4 in range(4):
                for dm in range(2):
                    tasks.append((("out", 3, t4, dm), g_out(3, t4, dm)))

            # ---- attention S->exp pipeline, draining fillers in PE slack
            def s_duo(qt, hp, j):
                sp = psp.tile([128, 1024], F32, name="sps", tag="sps", bufs=2)
                nc.tensor.matmul(
                    sp[:, 0:512], KT[hp][0:64, j * 128:(j + 1) * 128],
                    QT[hp][0:64, qt * 512:(qt + 1) * 512],
                    start=True, stop=True, tile_position=(0, 0))
                nc.tensor.matmul(
                    sp[:, 512:1024], KT[hp][64:128, j * 128:(j + 1) * 128],
                    QT[hp][64:128, qt * 512:(qt + 1) * 512],
                    start=True, stop=True, tile_position=(64, 0))
                return sp

            for qt in range(NQT):
                for hp in range(NHP):
                    if qt == 0 and hp > 0:
                        drain_until(("kt", hp, 3))
                    elif qt > 0:
                        drain_until(("qt", hp, qt))
                    s_cur = s_duo(qt, hp, 0)
                    for j in range(NJ):
                        # pt ring-slot reuse guard: the slot this exp writes
                        # must have its PV reads already EMITTED, else Tile
                        # can't order the WAR correctly.
                        g = 16 * (4 * qt + hp) + j
                        while pv_read_cnt[0] < g - PT_BUFS + 1:
                            assert tasks, "pt guard: no tasks left to drain"
                            drain(400)
                        pt = sb.tile([128, 1024], BF16, name="pt", tag="pt",
                                     bufs=PT_BUFS)
                        pt_tiles[(qt, hp, j)] = pt
                        nc.scalar.activation(pt[:], s_cur[:], AF.Exp,
                                             scale=0.125)
                        if j + 1 < NJ:
                            s_cur = s_duo(qt, hp, j + 1)
                        drain(BUDGET0 if qt == 0 else BUDGET)

            # ---- tail: whatever tasks remain
            while tasks:
                drain(1 << 30)
    nc.finalize()
    return nc


def kernel(x, Wq, Wk, Wv, Wo, bo, _trace=False):
    bf16 = ml_dtypes.bfloat16
    x = np.asarray(x, np.float32)
    bo = np.asarray(bo, np.float32)

    if "nc" not in _CACHE:
        _CACHE["nc"] = build()
    nc = _CACHE["nc"]

    ones_in = np.ones((128, 8), bf16)
    ident_in = np.eye(128, dtype=bf16)

    def cat128(w):
        # [n*128, c] -> [128, n*c] with k-tiles side by side
        w = np.asarray(w)
        n = w.shape[0] // 128
        return np.ascontiguousarray(
            np.concatenate([w[i * 128:(i + 1) * 128] for i in range(n)],
                           axis=1)).astype(bf16)

    in_maps = []
    for c in range(8):
        b, hb = c // 2, c % 2
        sl = slice(hb * 512, (hb + 1) * 512)
        in_maps.append({
            "xT": np.ascontiguousarray(x[b].T.astype(bf16)),
            "wq": cat128(np.asarray(Wq)[:, sl]),
            "wk": cat128(np.asarray(Wk)[:, sl]),
            "wv": cat128(np.asarray(Wv)[:, sl]),
            "wo": cat128(np.asarray(Wo)[sl, :]),
            "ones": ones_in, "ident": ident_in,
        })
    res = run_bass_kernel_spmd(nc, in_maps, list(range(8)), trace=_trace)
    out = np.empty((4, N, DIM), np.float32)
    for b in range(4):
        out[b] = res.results[2 * b]["out"] + res.results[2 * b + 1]["out"] + bo
    if _trace:
        return out, res
    return out
